# revision 1
# baseline (speedup 1.0000x reference)
"""Trainium2 Bass kernel for nn_CirculantSSMLayer.

y = WC @ real(ifft(scan(a_hat, u_hat))) + WC_b + D_skip * x
  a_hat = contract(fft(gamma * tanh(Wa @ LN(x))))     (fft over state dim, 256)
  u_hat = fft(WB @ LN(x))
  scan over time: h_t = a_hat_t * h_{t-1} + u_hat_t   (complex, per freq bin)

Key design points
-----------------
* Sharding: 8 cores = (4 batches) x (2 time-halves of 2048). The scan is
  magnitude-contracting (|a_hat| <= 0.5 by construction), so the second
  time-half recovers the scan state from a 32-step warmup prefix instead of
  cross-core communication: truncation error <= 0.5^32 ~ 2e-10.
* Everything runs transposed ([d, t] layout) so all contractions (LayerNorm
  stats, projections, FFT/IFFT as DFT matmuls) are partition-dim matmuls.
* Real inputs => conjugate-symmetric spectra: only bins 0..128 are needed.
  Bins 0..127 live on 128 partitions; the (real) Nyquist bin 128 rides in
  row 0 of the "imag" tiles (whose imag part is structurally zero) with a
  tiny 1-row side-path, rejoining via the DC column of the IFFT matrix.
* The complex scan is made REAL with a rotating frame: a = r*e^{i*phi},
  Phi = cumsum(phi) =>  g_t = r_t * g_{t-1} + u_t*e^{-i*Phi_t} runs as two
  tensor_tensor_scan instructions; h = g*e^{i*Phi}. phi comes from a
  half-angle atan2 (Arctan LUT), Phi is range-reduced mod pi with the
  Cody-Waite custom DVE op + round-via-magic-constant.
* LayerNorm is folded into the matmuls: proj(LN(x)) = (W@x - wsum x mu)*rs
  (+ bias); the rank-1 mean term is a K=1 matmul into the same PSUM group.
* Matmuls run as float32r (TF32-like, full PE rate, fp32 storage).
* D_skip * x + WC_b is added on the host during unsharding (free).
"""

import math
import sys
from contextlib import ExitStack

import numpy as np

for _p in ("/opt/trn_rl_repo",):
    if _p not in sys.path:
        sys.path.insert(0, _p)

import concourse.bacc as bacc
import concourse.bass as bass
import concourse.tile as tile
from concourse import mybir
from concourse.bass_utils import run_bass_kernel_spmd

B, T, D, NST = 4, 4096, 1024, 256
KB = 128            # spectral bins on partitions (0..127; Nyquist packed aside)
W = 32              # warmup columns
TC = 2048 + W       # per-core time columns
TOUT = 2048
KBLK = D // 128     # 8 contraction blocks over d
LN_EPS = 1e-5

F32 = mybir.dt.float32
F32R = mybir.dt.float32r
AF = mybir.ActivationFunctionType
OP = mybir.AluOpType

MAGIC = 12582912.0  # 1.5 * 2^23: add/sub forces round-to-nearest integer
PI = math.pi

# full-T matmul N-chunks and output (post-warmup) chunks
NCH = [(0, 512), (512, 512), (1024, 512), (1536, 512), (2048, W)]
YCH = [(0, 512), (512, 512), (1024, 512), (1536, 512)]

TRACE = False
LAST_RESULTS = None
_CACHE = {}

# scalar rows live at 32-aligned partitions (HW constraint on SBUF APs):
# rowsA: p0=var, p32=sd, p64=rs, p96=musq ; rowsB: p0=nyq_a(->nyq_r),
# p32=nyq_u(->nyq_g), p64=nyq_abs(->nyq_sig)


def _pi_triple():
    p = np.float64(np.pi)
    c1 = np.float32(np.trunc(p * 2**12) / 2**12)
    r = p - np.float64(c1)
    c2 = np.float32(np.trunc(r * 2**24) / 2**24)
    c3 = np.float32(p - np.float64(c1) - np.float64(c2))
    return float(c1), float(c2), float(c3)


def _build_nc():
    nc = bacc.Bacc("TRN2", target_bir_lowering=False, debug=False)

    def din(name, shape, dt=F32):
        return nc.dram_tensor(name, shape, dt, kind="ExternalInput")

    d_x = din("xT", [128, KBLK, TC], F32R)
    d_wa = din("wa", [128, KBLK, NST], F32R)
    d_wufre = din("wufre", [128, KBLK, KB], F32R)
    d_wufim = din("wufim", [128, KBLK, KB], F32R)
    d_fre = din("fre", [128, 2, KB], F32R)
    d_fim = din("fim", [128, 2, KB], F32R)
    d_icre = din("icre", [128, NST], F32R)
    d_icim = din("icim", [128, NST], F32R)
    d_wc = din("wc", [128, 2, D], F32R)
    d_wasum = din("wasum_neg", [1, NST], F32R)
    d_fwre = din("fwre_neg", [1, KB], F32R)
    d_fwim = din("fwim_neg", [1, KB], F32R)
    d_fbre = din("fbre", [128, 1])
    d_fbim = din("fbim", [128, 1])
    d_abias = din("abias", [128, 2])
    d_wmask = din("wmask", [128, W])
    d_ones = din("ones_col", [128, 1], F32R)
    d_y = nc.dram_tensor("yT", [D, TOUT], F32, kind="ExternalOutput")

    pc1, pc2, pc3 = _pi_triple()
    import itertools
    _ctr = itertools.count()

    with tile.TileContext(nc) as tc, ExitStack() as ctx:
        v = nc.vector
        sc_e = nc.scalar

        # scalar bias/scale values used by activation() must exist as const APs
        for _cv in (LN_EPS, 2.0, -2.0, 4.0):
            _ct = nc.alloc_sbuf_tensor(f"constf32-{_cv}", [128, 1], F32)
            nc.gpsimd.memset(_ct.ap(), _cv)
            nc.const_aps.aps[(F32, _cv)] = _ct.ap()

        # ---- long-lived pools (entered first: survive the whole kernel)
        wp = ctx.enter_context(tc.tile_pool(name="weights", bufs=1))
        rows = ctx.enter_context(tc.tile_pool(name="rows", bufs=1))
        big = ctx.enter_context(tc.tile_pool(name="big", bufs=1))

        # inputs/weights to SBUF
        x_s = wp.tile([128, KBLK, TC], F32R)
        nc.sync.dma_start(x_s[:], d_x[:])
        wa_s = wp.tile([128, KBLK, NST], F32R)
        nc.sync.dma_start(wa_s[:], d_wa[:])
        wufre_s = wp.tile([128, KBLK, KB], F32R)
        nc.sync.dma_start(wufre_s[:], d_wufre[:])
        wufim_s = wp.tile([128, KBLK, KB], F32R)
        nc.sync.dma_start(wufim_s[:], d_wufim[:])
        fre_s = wp.tile([128, 2, KB], F32R)
        nc.sync.dma_start(fre_s[:], d_fre[:])
        fim_s = wp.tile([128, 2, KB], F32R)
        nc.sync.dma_start(fim_s[:], d_fim[:])
        icre_s = wp.tile([128, NST], F32R)
        nc.sync.dma_start(icre_s[:], d_icre[:])
        icim_s = wp.tile([128, NST], F32R)
        nc.sync.dma_start(icim_s[:], d_icim[:])
        wc_s = wp.tile([128, 2, D], F32R)
        nc.sync.dma_start(wc_s[:], d_wc[:])
        wasum_s = rows.tile([1, NST], F32R)
        nc.sync.dma_start(wasum_s[:], d_wasum[:])
        fwre_s = rows.tile([1, KB], F32R)
        nc.sync.dma_start(fwre_s[:], d_fwre[:])
        fwim_s = rows.tile([1, KB], F32R)
        nc.sync.dma_start(fwim_s[:], d_fwim[:])
        fbre_s = rows.tile([128, 1], F32)
        nc.sync.dma_start(fbre_s[:], d_fbre[:])
        fbim_s = rows.tile([128, 1], F32)
        nc.sync.dma_start(fbim_s[:], d_fbim[:])
        abias_s = rows.tile([128, 2], F32)
        nc.sync.dma_start(abias_s[:], d_abias[:])
        wmask_s = rows.tile([128, W], F32)
        nc.sync.dma_start(wmask_s[:], d_wmask[:])

        ones_col = rows.tile([128, 1], F32R)   # K=128, M=1 lhsT for stats sums
        nc.sync.dma_start(ones_col[:], d_ones[:])

        # scalar rows: every DVE/ACT op needs all SBUF operands at the SAME
        # base partition, so all rows live at partition 0 of distinct tiles
        # (including dead partition-0 rows of big tiles; lifetimes disjoint).
        rwA = rows.tile([128, TC], F32)
        rwB = rows.tile([128, TC], F32)
        rwC = rows.tile([1, TC], F32)
        rowA = rwA[0:1, :]      # var -> rs -> (later) nyq_a/nyq_r
        rowB = rwB[0:1, :]      # musq/scratch -> (later) nyq_u/nyq_g
        rowC = rwC[0:1, :]      # |nyq_a| -> sigmoid (in place)

        # persistent [128, TC]-class tiles; tags chain disjoint lifetimes
        mu_t = big.tile([1, TC], F32R, tag="rp")       # -> rprime later
        RS_b = big.tile([128, TC], F32, tag="slotC")  # -> ahre later
        apre0 = big.tile([128, TC], F32R, tag="slotA")
        apre1 = big.tile([128, TC], F32R, tag="slotB")
        u_re = big.tile([128, TC], F32, tag="slotE")
        u_im = big.tile([128, TC], F32, tag="slotF")

        # ---------------- stats: sx = sum_d x, sx2 = sum_d x^2 ------------
        with tc.tile_pool(name="statp", bufs=2, space="PSUM") as pstat, \
             tc.tile_pool(name="statsq", bufs=3) as sqp:
            for (c0, cw) in NCH:
                ps1 = pstat.tile([1, 512], F32, tag="sx",
                                 name=f"ps1_{next(_ctr)}")
                ps2 = pstat.tile([1, 512], F32, tag="sx2",
                                 name=f"ps2_{next(_ctr)}")
                for kb in range(KBLK):
                    xs = x_s[:, kb, c0:c0 + cw]
                    sq = sqp.tile([128, 512], F32R, tag="sq",
                                  name=f"sq_{next(_ctr)}")
                    sc_e.activation(sq[:, :cw], xs.bitcast(F32), AF.Square)
                    nc.tensor.matmul(ps1[:, :cw], (ones_col[:]), (xs),
                                     start=(kb == 0), stop=(kb == KBLK - 1))
                    nc.tensor.matmul(ps2[:, :cw], (ones_col[:]),
                                     (sq[:, :cw]),
                                     start=(kb == 0), stop=(kb == KBLK - 1))
                v.tensor_scalar(out=mu_t[:, c0:c0 + cw], in0=ps1[:, :cw],
                                scalar1=1.0 / D, scalar2=None, op0=OP.mult)
                v.tensor_scalar(out=rowA[:, c0:c0 + cw], in0=ps2[:, :cw],
                                scalar1=1.0 / D, scalar2=None, op0=OP.mult)

        # var = E[x^2] - mu^2 ; sd = sqrt(var+eps) ; rs = 1/sd
        sdrow = RS_b[0:1, :]   # RS_b row 0 is dead until the broadcast DMA
        v.tensor_mul(rowB, mu_t[:].bitcast(F32), mu_t[:].bitcast(F32))
        v.tensor_sub(rowA, rowA, rowB)
        sc_e.activation(sdrow, rowA, AF.Sqrt, bias=LN_EPS)
        v.reciprocal_approx_accurate(out=rowA, in_=sdrow, scratch=rowB)

        # broadcast rs across partitions: bounce through DRAM, then load with
        # a zero-stride (broadcast) DRAM source AP
        d_rs = nc.dram_tensor("rs_scratch", [1, TC], F32)
        nc.sync.dma_start(d_rs[:], rowA)
        rs_dram = d_rs[:]
        rs_bcast = bass.AP(tensor=rs_dram.tensor, offset=rs_dram.offset,
                           ap=[[0, 128], [1, TC]])
        nc.sync.dma_start(RS_b[:], rs_bcast)

        tmp = ctx.enter_context(tc.tile_pool(name="tmpT", bufs=3))
        pp = ctx.enter_context(tc.tile_pool(name="mmp", bufs=5, space="PSUM"))

        def mmps():
            return pp.tile([128, 512], F32, tag="mm", name=f"mm_{next(_ctr)}")

        def tmpt(nm):
            return tmp.tile([128, TC], F32, tag="t", name=f"{nm}_{next(_ctr)}")

        # ---------------- proj_a -> tanh -> a_pre -------------------------
        for m, apre in ((0, apre0), (1, apre1)):
            msl = slice(m * 128, (m + 1) * 128)
            psums = [mmps() for _ in NCH]
            for kb in range(KBLK):
                for ci, (c0, cw) in enumerate(NCH):
                    nc.tensor.matmul(psums[ci][:, :cw],
                                     (wa_s[:, kb, msl]),
                                     (x_s[:, kb, c0:c0 + cw]),
                                     start=(kb == 0), stop=False)
            praw = tmpt("praw")
            for ci, (c0, cw) in enumerate(NCH):
                # rank-1 mean correction: += (-wasum_m) (x) mu
                nc.tensor.matmul(psums[ci][:, :cw],
                                 (wasum_s[:, msl]),
                                 (mu_t[:, c0:c0 + cw]),
                                 start=False, stop=True)
                v.tensor_mul(praw[:, c0:c0 + cw], psums[ci][:, :cw],
                             RS_b[:, c0:c0 + cw])
            sc_e.activation(apre[:], praw[:], AF.Tanh, bias=abias_s[:, m:m + 1])

        # ---------------- u_hat (FFT folded into WB projection) -----------
        for wuf, fwn, fbn, udst in ((wufre_s, fwre_s, fbre_s, u_re),
                                    (wufim_s, fwim_s, fbim_s, u_im)):
            psums = [mmps() for _ in NCH]
            for kb in range(KBLK):
                for ci, (c0, cw) in enumerate(NCH):
                    nc.tensor.matmul(psums[ci][:, :cw],
                                     (wuf[:, kb, :]),
                                     (x_s[:, kb, c0:c0 + cw]),
                                     start=(kb == 0), stop=False)
            for ci, (c0, cw) in enumerate(NCH):
                nc.tensor.matmul(psums[ci][:, :cw], (fwn[:]),
                                 (mu_t[:, c0:c0 + cw]), start=False,
                                 stop=True)
                v.tensor_mul(udst[:, c0:c0 + cw], psums[ci][:, :cw],
                             RS_b[:, c0:c0 + cw])
            # per-partition fourier bias (fb = F @ u_bias)
            v.tensor_scalar(out=udst[:], in0=udst[:], scalar1=fbn[:, 0:1],
                            scalar2=None, op0=OP.add)

        # warmup masking of u, then peel off the Nyquist row
        v.tensor_mul(u_re[:, :W], u_re[:, :W], wmask_s[:])
        v.tensor_mul(u_im[:, :W], u_im[:, :W], wmask_s[:])
        sc_e.copy(rowB, u_im[0:1, :])
        nc.gpsimd.memset(u_im[0:1, :], 0.0)

        # ---------------- FFT of a (DFT matmul over state dim) ------------
        ahre = big.tile([128, TC], F32, tag="slotC")  # reuses RS_b slot
        ahim = big.tile([128, TC], F32, tag="slotD")
        for fmat, adst in ((fre_s, ahre), (fim_s, ahim)):
            psums = [mmps() for _ in NCH]
            for kq, apre in ((0, apre0), (1, apre1)):
                for ci, (c0, cw) in enumerate(NCH):
                    nc.tensor.matmul(psums[ci][:, :cw],
                                     (fmat[:, kq, :]),
                                     (apre[:, c0:c0 + cw]),
                                     start=(kq == 0), stop=(kq == 1))
            for ci, (c0, cw) in enumerate(NCH):
                sc_e.copy(adst[:, c0:c0 + cw], psums[ci][:, :cw])

        v.tensor_mul(ahre[:, :W], ahre[:, :W], wmask_s[:])
        v.tensor_mul(ahim[:, :W], ahim[:, :W], wmask_s[:])
        sc_e.copy(rowA, ahim[0:1, :])
        nc.gpsimd.memset(ahim[0:1, :], 0.0)

        # ---------------- magnitude, contraction scale, phase -------------
        sqre = tmpt("sqre")
        sc_e.activation(sqre[:], ahre[:], AF.Square)
        sqim = tmpt("sqim")
        sc_e.activation(sqim[:], ahim[:], AF.Square)
        v.tensor_add(sqre[:], sqre[:], sqim[:])          # mag^2 (in place)
        r_t = tmpt("r_t")
        sc_e.activation(r_t[:], sqre[:], AF.Sqrt)        # r = |a_hat|
        sc_e.activation(rowC, rowA, AF.Abs)

        sig = tmpt("sig")
        sc_e.activation(sig[:], r_t[:], AF.Sigmoid, scale=-2.0, bias=2.0)
        sc_e.activation(rowC, rowC, AF.Sigmoid, scale=-2.0, bias=2.0)
        rprime = big.tile([128, TC], F32, tag="rp")      # reuses mu slot
        v.tensor_mul(rprime[:], r_t[:], sig[:])          # scan coefficient
        v.tensor_mul(rowA, rowA, rowC)         # signed real coeff (in place)

        # half-angle atan2: phi/2 = atan((im + e1) / (r + re + e2))
        den = tmpt("den")
        v.tensor_add(den[:], r_t[:], ahre[:])
        # r + re cancels to exactly 0 on the negative real axis; clamp after
        v.tensor_scalar(out=den[:], in0=den[:], scalar1=1e-30, scalar2=None,
                        op0=OP.max)
        # quarter-angle: tan(phi/4) = aim / (rho + r + re), rho^2 = 2 r (r+re)
        v.tensor_mul(r_t[:], r_t[:], den[:])             # r*den (in place)
        sc_e.activation(r_t[:], r_t[:], AF.Sqrt, scale=2.0)   # rho
        v.tensor_add(den[:], r_t[:], den[:])             # den4 (in place)
        v.reciprocal_approx_fast(out=den[:], in_=den[:])  # 1/den4 (in place)
        q = tmpt("q")
        v.scalar_tensor_tensor(out=q[:], in0=ahim[:], scalar=1e-11,
                               in1=den[:], op0=OP.add, op1=OP.mult)
        v.tensor_scalar(out=q[:], in0=q[:], scalar1=1.0, scalar2=-1.0,
                        op0=OP.min, op1=OP.max)
        at = tmpt("at")
        sc_e.activation(at[:], q[:], AF.Arctan)          # phi/4 in [-pi/4,pi/4]

        # Phi/4 = cumsum(phi/4); reduce mod pi/2; sins of the 4x angle
        ones_bc = nc.const_aps.tensor(1.0, (128, TC))
        ph = tmpt("ph")
        v.tensor_tensor_scan(out=ph[:], data0=ones_bc, data1=at[:],
                             initial=0.0, op0=OP.mult, op1=OP.add)
        kq_t = tmpt("kq")
        v.tensor_scalar(out=kq_t[:], in0=ph[:], scalar1=2.0 / PI,
                        scalar2=MAGIC, op0=OP.mult, op1=OP.add)
        v.tensor_scalar(out=kq_t[:], in0=kq_t[:], scalar1=MAGIC, scalar2=None,
                        op0=OP.subtract)
        phr = tmpt("phr")
        v.cody_waite_cascade(out=phr[:], x=ph[:], k=kq_t[:], c1=pc1 / 2,
                             c2=pc2 / 2, c3=pc3 / 2)
        # keep 4*angle strictly inside the Sin LUT range [-pi, pi]
        QB = 0.785398
        v.tensor_scalar(out=phr[:], in0=phr[:], scalar1=QB, scalar2=-QB,
                        op0=OP.min, op1=OP.max)
        carg = tmpt("carg")
        v.add_range_wrap(out=carg[:], in_=phr[:], shift=PI / 8, bound=PI / 4,
                         period=PI / 2)
        v.tensor_scalar(out=carg[:], in0=carg[:], scalar1=QB, scalar2=-QB,
                        op0=OP.min, op1=OP.max)
        s_t = big.tile([128, TC], F32, tag="slotA")      # reuses apre0 slot
        sc_e.activation(s_t[:], phr[:], AF.Sin, scale=4.0)   # sin(Phi)
        c_t = big.tile([128, TC], F32, tag="slotB")      # reuses apre1 slot
        sc_e.activation(c_t[:], carg[:], AF.Sin, scale=4.0)  # cos(Phi)

        # ---------------- rotate u, scan, rotate back ---------------------
        m1 = tmpt("m1")
        v.tensor_mul(m1[:], u_re[:], c_t[:])
        m4 = tmpt("m4")
        v.tensor_mul(m4[:], u_re[:], s_t[:])
        m2 = tmpt("m2")
        v.tensor_mul(m2[:], u_im[:], s_t[:])
        w_re = u_re
        v.tensor_add(w_re[:], m1[:], m2[:])              # u_re*c + u_im*s
        m3 = tmpt("m3")
        v.tensor_mul(m3[:], u_im[:], c_t[:])
        w_im = u_im
        v.tensor_sub(w_im[:], m3[:], m4[:])              # u_im*c - u_re*s

        v.tensor_tensor_scan(out=w_re[:], data0=rprime[:], data1=w_re[:],
                             initial=0.0, op0=OP.mult, op1=OP.add)
        v.tensor_tensor_scan(out=w_im[:], data0=rprime[:], data1=w_im[:],
                             initial=0.0, op0=OP.mult, op1=OP.add)
        v.tensor_tensor_scan(out=rowB, data0=rowA, data1=rowB,
                             initial=0.0, op0=OP.mult, op1=OP.add)

        # h = g * e^{+i Phi}, only for the kept (post-warmup) columns
        g_re, g_im = w_re, w_im
        ko = slice(W, TC)
        n1 = tmpt("n1")
        v.tensor_mul(n1[:, :TOUT], g_re[:, ko], c_t[:, ko])
        n2 = tmpt("n2")
        v.tensor_mul(n2[:, :TOUT], g_im[:, ko], s_t[:, ko])
        n4 = tmpt("n4")
        v.tensor_mul(n4[:, :TOUT], g_re[:, ko], s_t[:, ko])
        h_re = big.tile([128, TC], F32R, tag="slotE")     # reuses g_re slot
        v.tensor_sub(h_re[:, :TOUT], n1[:, :TOUT], n2[:, :TOUT])
        n3 = tmpt("n3")
        v.tensor_mul(n3[:, :TOUT], g_im[:, ko], c_t[:, ko])
        h_im = big.tile([128, TC], F32R, tag="slotF")     # reuses g_im slot
        v.tensor_add(h_im[:, :TOUT], n3[:, :TOUT], n4[:, :TOUT])
        # Nyquist h rides the (otherwise zero-weighted) DC column of icim
        sc_e.copy(h_im[0:1, :TOUT], rowB[:, W:])

        # ---------------- IRFFT + WC output matmuls -----------------------
        with tc.tile_pool(name="htp", bufs=1) as htp, \
             tc.tile_pool(name="yp", bufs=1) as yp:
            for ci, (c0, cw) in enumerate(YCH):
                hts = []
                for m2 in range(2):
                    msl = slice(m2 * 128, (m2 + 1) * 128)
                    psh = mmps()
                    nc.tensor.matmul(psh[:, :cw], (icre_s[:, msl]),
                                     (h_re[:, c0:c0 + cw]),
                                     start=True, stop=False)
                    nc.tensor.matmul(psh[:, :cw], (icim_s[:, msl]),
                                     (h_im[:, c0:c0 + cw]),
                                     start=False, stop=True)
                    ht = htp.tile([128, 512], F32R, tag=f"ht{m2}",
                                  name=f"ht{m2}_{next(_ctr)}")
                    sc_e.copy(ht[:, :cw], psh[:, :cw])
                    hts.append(ht)
                for m in range(KBLK):
                    msl = slice(m * 128, (m + 1) * 128)
                    psy = mmps()
                    for kq in range(2):
                        nc.tensor.matmul(psy[:, :cw],
                                         (wc_s[:, kq, msl]),
                                         (hts[kq][:, :cw]),
                                         start=(kq == 0), stop=(kq == 1))
                    ysb = yp.tile([128, 512], F32, tag="y",
                                  name=f"y_{next(_ctr)}")
                    nc.any.tensor_copy(ysb[:, :cw], psy[:, :cw])
                    nc.sync.dma_start(d_y[msl, c0:c0 + cw], ysb[:, :cw])

    nc.compile()
    return nc


def _get_nc():
    if "nc" not in _CACHE:
        _CACHE["nc"] = _build_nc()
    return _CACHE["nc"]


def _pack_lhsT(a):
    """[K, M] (K multiple of 128) -> [128, K//128, M] partition packing."""
    K, M = a.shape
    return np.ascontiguousarray(
        a.reshape(K // 128, 128, M).transpose(1, 0, 2)).astype(np.float32)


def _host_weights(inputs):
    f8 = np.float64
    lnw = np.asarray(inputs["ln_w"], f8)
    lnb = np.asarray(inputs["ln_b"], f8)
    Wa_w = np.asarray(inputs["Wa_w"], f8)
    Wa_b = np.asarray(inputs["Wa_b"], f8)
    WB_w = np.asarray(inputs["WB_w"], f8)
    WB_b = np.asarray(inputs["WB_b"], f8)
    WC_w = np.asarray(inputs["WC_w"], f8)
    log_gamma = float(np.asarray(inputs["log_gamma"], f8))
    gamma = 1.0 / (1.0 + math.exp(-log_gamma))

    Wa = Wa_w * lnw[None, :]                      # [256, 1024]
    abias = Wa_b + Wa_w @ lnb                     # [256]
    WBe = WB_w * lnw[None, :]
    bu = WB_b + WB_w @ lnb

    jj = np.arange(NST, dtype=f8)
    kk = np.arange(KB, dtype=f8)
    th = 2.0 * np.pi * np.outer(kk, jj) / NST     # [128, 256]
    G_re = np.cos(th)
    G_im = -np.sin(th)
    G_im[0, :] = (-1.0) ** jj                     # Nyquist(real) in im row 0
    F_re = gamma * G_re
    F_im = gamma * G_im

    WuF_re = G_re @ WBe                           # [128, 1024]
    WuF_im = G_im @ WBe
    fb_re = G_re @ bu
    fb_im = G_im @ bu

    thi = 2.0 * np.pi * np.outer(jj, kk) / NST    # [256, 128]
    ICre = (2.0 - (kk[None, :] == 0)) / NST * np.cos(thi)
    ICim = -2.0 / NST * np.sin(thi)
    ICim[:, 0] = ((-1.0) ** jj) / NST             # Nyquist via h_im DC column

    wts = {
        "wa": _pack_lhsT(Wa.T),
        "wufre": _pack_lhsT(WuF_re.T),
        "wufim": _pack_lhsT(WuF_im.T),
        "fre": _pack_lhsT(F_re.T),
        "fim": _pack_lhsT(F_im.T),
        "icre": np.ascontiguousarray(ICre.T).astype(np.float32),
        "icim": np.ascontiguousarray(ICim.T).astype(np.float32),
        "wc": _pack_lhsT(WC_w.T),
        "wasum_neg": (-Wa.sum(1))[None, :].astype(np.float32),
        "fwre_neg": (-WuF_re.sum(1))[None, :].astype(np.float32),
        "fwim_neg": (-WuF_im.sum(1))[None, :].astype(np.float32),
        "fbre": fb_re[:, None].astype(np.float32),
        "fbim": fb_im[:, None].astype(np.float32),
        "ones_col": np.ones((128, 1), np.float32),
        "abias": np.ascontiguousarray(
            abias.reshape(2, 128).T).astype(np.float32),
    }
    return {k: np.ascontiguousarray(v) for k, v in wts.items()}


def make_in_maps(inputs):
    x = np.asarray(inputs["x"], np.float32)
    wts = _host_weights(inputs)
    in_maps = []
    for c in range(8):
        b, half = divmod(c, 2)
        t0 = half * TOUT
        xs = np.zeros((D, TC), np.float32)
        if t0 - W < 0:
            xs[:, W:] = x[b, 0:TOUT, :].T
            wm = np.zeros((128, W), np.float32)
        else:
            xs[:, :] = x[b, t0 - W:t0 + TOUT, :].T
            wm = np.ones((128, W), np.float32)
        m = dict(wts)
        m["xT"] = np.ascontiguousarray(
            xs.reshape(KBLK, 128, TC).transpose(1, 0, 2))
        m["wmask"] = wm
        in_maps.append(m)
    return in_maps


def kernel(**inputs):
    global LAST_RESULTS
    x = np.asarray(inputs["x"], np.float32)
    D_skip = np.asarray(inputs["D_skip"], np.float32)
    WC_b = np.asarray(inputs["WC_b"], np.float32)

    nc = _get_nc()
    in_maps = make_in_maps(inputs)
    res = run_bass_kernel_spmd(nc, in_maps, core_ids=list(range(8)),
                               trace=TRACE)
    LAST_RESULTS = res

    y = np.empty((B, T, D), np.float32)
    for c in range(8):
        b, half = divmod(c, 2)
        t0 = half * TOUT
        y[b, t0:t0 + TOUT, :] = res.results[c]["yT"].T
    y += x * D_skip[None, None, :] + WC_b[None, None, :]
    return y

